# revision 25
# baseline (speedup 1.0000x reference)
"""Trainium2 Bass kernel for conv-projected multi-head attention.

Reference computation (per batch item b of 8, one NeuronCore each):
  y   = BN(depthwise3x3(x_b reshaped to [C,32,32]))      # q = k = v = y
  q/k/v = y @ w{q,k,v}^T  (heads: 12 x 32)
  att = softmax((q @ k^T) * sqrt(32))
  out = (att @ v) @ wo^T

Device layout is channel-major ("transposed"): xT [C=384, T=1024].
 - conv: 9 accumulating diag-matmuls on PE over a zero-padded [34x34] image
 - qT/kT [o, t] via lhsT=w^T; v kept [t, o] (augmented with a ones column
   so the PV matmul also yields the softmax denominators)
 - scores S^T[t, l] per head with K=32 contraction, 4 heads row-packed in
   the PE array via tile_position
 - exp on ACT directly from a 4-bank PSUM tile, output fp32r
 - PV: lhsT = vaug [t,34], rhs = E [t,l]; out rows 0-31 = O^T, row 32 = sums
 - normalize after PV (per-head reciprocal broadcast), concat, out-proj
All matmuls in float32r (~2e-4 rel err, full PE rate).
"""
import sys

sys.path.insert(0, "/opt/trn_rl_repo")
from contextlib import ExitStack

import numpy as np

B, T, C = 8, 1024, 384
NH, DH = 12, 32
HH = WW = 32
SCALE = float(DH) ** 0.5
BN_EPS = 1e-5
NCORES = 8

_CACHE = {}


def _build(debug=False, stage=5):
    import concourse.bass as bass
    import concourse.tile as tile
    from concourse import bacc, mybir
    from concourse.masks import make_identity

    F32 = mybir.dt.float32
    F32R = mybir.dt.float32r
    AF = mybir.ActivationFunctionType
    ALU = mybir.AluOpType

    nc = bacc.Bacc("TRN2", target_bir_lowering=False, debug=False)

    xt_d = nc.dram_tensor("xt", [C, T], F32R, kind="ExternalInput").ap()
    w9_d = nc.dram_tensor("w9", [C, 9], F32, kind="ExternalInput").ap()
    bias_d = nc.dram_tensor("bias", [C, 1], F32, kind="ExternalInput").ap()
    wqT_d = nc.dram_tensor("wqT", [C, C], F32R, kind="ExternalInput").ap()
    wkT_d = nc.dram_tensor("wkT", [C, C], F32R, kind="ExternalInput").ap()
    wvT_d = nc.dram_tensor("wvT", [C, C], F32R, kind="ExternalInput").ap()
    woT_d = nc.dram_tensor("woT", [C, C], F32R, kind="ExternalInput").ap()
    outT_d = nc.dram_tensor("outT", [C, T], F32, kind="ExternalOutput").ap()
    dbg = {}
    if debug:
        dbg["y"] = nc.dram_tensor("dbg_y", [C, T], F32, kind="ExternalOutput").ap()
        dbg["qT"] = nc.dram_tensor("dbg_qT", [C, T], F32, kind="ExternalOutput").ap()
        dbg["vaug"] = nc.dram_tensor(
            "dbg_vaug", [T, NH * 34], F32, kind="ExternalOutput"
        ).ap()
        dbg["E0"] = nc.dram_tensor(
            "dbg_E0", [2, T, 4 * 512], F32, kind="ExternalOutput"
        ).ap()
        dbg["attn"] = nc.dram_tensor(
            "dbg_attn", [C, T], F32, kind="ExternalOutput"
        ).ap()
        dbg["ov"] = nc.dram_tensor(
            "dbg_ov", [4, 34, T], F32, kind="ExternalOutput"
        ).ap()
        dbg["R"] = nc.dram_tensor("dbg_R", [128, T], F32, kind="ExternalOutput").ap()

    CT = C // 128  # 3 c-tiles
    TT = T // 128  # 8 t-tiles
    TH = T // 512  # 2 t-halves / l-halves

    with tile.TileContext(nc) as tc, ExitStack() as top:
        # ---- persistent pools ----
        persist = top.enter_context(tc.tile_pool(name="persist", bufs=1))
        copies = top.enter_context(tc.tile_pool(name="copies", bufs=3))

        # persistent SBUF tensors
        y_sb = [persist.tile([128, T], F32R, tag=f"y{i}", name=f"y{i}") for i in range(CT)]
        qT_sb = [persist.tile([128, T], F32R, tag=f"q{i}", name=f"q{i}") for i in range(CT)]
        kT_sb = [persist.tile([128, T], F32R, tag=f"k{i}", name=f"k{i}") for i in range(CT)]
        vaug = [persist.tile([128, NH, 34], F32R, tag=f"va{i}", name=f"va{i}") for i in range(TT)]
        attn_sb = [persist.tile([128, T], F32R, tag=f"at{i}", name=f"at{i}") for i in range(CT)] if stage >= 4 else None
        wT_sb = {}
        for nm, d in (("q", wqT_d), ("k", wkT_d), ("v", wvT_d), ("o", woT_d)):
            tiles = [persist.tile([128, C], F32R, tag=f"w{nm}{i}", name=f"w{nm}{i}") for i in range(CT)]
            for i in range(CT):
                nc.sync.dma_start(tiles[i][:], d[i * 128 : (i + 1) * 128, :])
            wT_sb[nm] = tiles

        bias_sb = [persist.tile([128, 1], F32, tag=f"b{i}", name=f"b{i}") for i in range(CT)]
        for i in range(CT):
            nc.sync.dma_start(bias_sb[i][:], bias_d[i * 128 : (i + 1) * 128, :])

        ones1 = persist.tile([128, 32], F32R, tag="ones1")
        nc.vector.memset(ones1[:].bitcast(F32), 1.0)

        with ExitStack() as ph1:
            convpool = ph1.enter_context(tc.tile_pool(name="convpool", bufs=1))
            conv_ps = ph1.enter_context(
                tc.tile_pool(name="conv_ps", bufs=2, space="PSUM")
            )
            qk_ps = ph1.enter_context(tc.tile_pool(name="qk_ps", bufs=4, space="PSUM"))
            v_ps = ph1.enter_context(tc.tile_pool(name="v_ps", bufs=2, space="PSUM"))

            # ---- padded input and diag weights ----
            xp = [convpool.tile([128, 34 * 34], F32R, tag=f"xp{i}", name=f"xp{i}") for i in range(CT)]
            w9_sb = [convpool.tile([128, 9], F32, tag=f"w9{i}", name=f"w9s{i}") for i in range(CT)]
            ident = convpool.tile([128, 128], F32, tag="ident")
            diag = [convpool.tile([128, 9, 128], F32R, tag=f"dg{i}", name=f"dg{i}") for i in range(CT)]

            make_identity(nc, ident[:])
            for i in range(CT):
                nc.vector.memset(xp[i][:].bitcast(F32), 0.0)
                nc.sync.dma_start(
                    xp[i][:].rearrange("p (a b) -> p a b", a=34)[:, 1:33, 1:33],
                    xt_d[i * 128 : (i + 1) * 128, :].rearrange(
                        "p (a b) -> p a b", a=32
                    ),
                )
                nc.sync.dma_start(w9_sb[i][:], w9_d[i * 128 : (i + 1) * 128, :])
                for k in range(9):
                    nc.vector.tensor_scalar_mul(
                        diag[i][:, k, :], ident[:], w9_sb[i][:, k : k + 1]
                    )

            # ---- conv: 9 accumulating diag matmuls per (c-tile, t-half) ----
            for i in range(CT):
                for th in range(TH):
                    yp = conv_ps.tile([128, 512], F32, tag="conv")
                    r0 = th * 16  # image-row offset of this half
                    for k in range(9):
                        dy, dx = k // 3 - 1, k % 3 - 1
                        off = (r0 + 1 + dy) * 34 + (1 + dx)
                        rhs = bass.AP(
                            tensor=xp[i].tensor,
                            offset=xp[i].offset + off,
                            ap=[list(p) for p in xp[i].ap[:1]] + [[34, 16], [1, 32]],
                        )
                        nc.tensor.matmul(
                            yp[:].rearrange("p (a b) -> p a b", a=16),
                            diag[i][:, k, :],
                            rhs,
                            start=(k == 0),
                            stop=(k == 8),
                        )
                    # + BN bias, round to f32r, store to y
                    nc.vector.tensor_scalar_add(
                        y_sb[i][:, th * 512 : (th + 1) * 512],
                        yp[:],
                        bias_sb[i][:],
                    )
            if debug:
                for i in range(CT):
                    nc.sync.dma_start(
                        dbg["y"][i * 128 : (i + 1) * 128, :], y_sb[i][:].bitcast(F32)
                    )

            # ---- q/k projections: qT[o, t] ----
            for nm, dst in (("q", qT_sb), ("k", kT_sb)):
                for ot in range(CT):
                    for th in range(TH):
                        pp = qk_ps.tile([128, 512], F32, tag="qk")
                        for kt in range(CT):
                            nc.tensor.matmul(
                                pp[:],
                                wT_sb[nm][kt][:, ot * 128 : (ot + 1) * 128],
                                y_sb[kt][:, th * 512 : (th + 1) * 512],
                                start=(kt == 0),
                                stop=(kt == CT - 1),
                            )
                        nc.vector.tensor_copy(
                            dst[ot][:, th * 512 : (th + 1) * 512], pp[:]
                        )
            if debug:
                for i in range(CT):
                    nc.sync.dma_start(
                        dbg["qT"][i * 128 : (i + 1) * 128, :], qT_sb[i][:].bitcast(F32)
                    )

            # ---- v projection: v[t, o], written into vaug [t, h, 34] ----
            for tt in range(TT):
                vp = v_ps.tile([128, C], F32, tag="v")
                for kt in range(CT):
                    nc.tensor.matmul(
                        vp[:],
                        y_sb[kt][:, tt * 128 : (tt + 1) * 128],
                        wT_sb["v"][kt][:],
                        start=(kt == 0),
                        stop=(kt == CT - 1),
                    )
                nc.vector.memset(vaug[tt][:, :, 32:34].bitcast(F32), 0.0)
                nc.vector.memset(vaug[tt][:, :, 32:33].bitcast(F32), 1.0)
                nc.vector.tensor_copy(
                    vaug[tt][:, :, 0:32],
                    vp[:].rearrange("p (h d) -> p h d", h=NH),
                )
            if debug:
                for tt in range(TT):
                    nc.sync.dma_start(
                        dbg["vaug"][tt * 128 : (tt + 1) * 128, :],
                        vaug[tt][:].bitcast(F32).rearrange("p a b -> p (a b)"),
                    )

        # ---- attention ----
        with ExitStack() as ph2:
            s_ps = ph2.enter_context(tc.tile_pool(name="s_ps", bufs=1, space="PSUM"))
            ov_ps = ph2.enter_context(tc.tile_pool(name="ov_ps", bufs=2, space="PSUM"))
            sg_ps = ph2.enter_context(tc.tile_pool(name="sg_ps", bufs=1, space="PSUM"))
            rb_ps = ph2.enter_context(tc.tile_pool(name="rb_ps", bufs=1, space="PSUM"))
            epool = ph2.enter_context(tc.tile_pool(name="epool", bufs=10))
            rpool = ph2.enter_context(tc.tile_pool(name="rpool", bufs=3))
            ovpool = ph2.enter_context(tc.tile_pool(name="ovpool", bufs=6))

            # indicator tiles: ind4[j][k, m] = 1.0 iff (k, m) == (32, 32*j);
            # a gather matmul with this lhsT moves a head's sums row (psum
            # partition 32) to aligned partition 32*j of the gather tile.
            ind4 = []
            for j in range(4):
                it = rpool.tile([34, 128], F32R, tag=f"ind{j}", name=f"ind{j}")
                nc.vector.memset(it[:].bitcast(F32), 0.0)
                nc.vector.memset(
                    it[32:33, 32 * j : 32 * j + 1].bitcast(F32), 1.0
                )
                ind4.append(it)

            for lh in range(TH):
                for g in range(CT):
                    E = []
                    for tt in range(TT):
                        s4 = s_ps.tile([128, 2048], F32, tag="s4")
                        for hh in range(4):
                            nc.tensor.matmul(
                                s4[:, 512 * hh : 512 * (hh + 1)],
                                kT_sb[g][
                                    32 * hh : 32 * (hh + 1),
                                    tt * 128 : (tt + 1) * 128,
                                ],
                                qT_sb[g][
                                    32 * hh : 32 * (hh + 1),
                                    lh * 512 : (lh + 1) * 512,
                                ],
                                start=True,
                                stop=True,
                                tile_position=(32 * hh, 0),
                            )
                        e = epool.tile([128, 2048], F32R, tag="E")
                        nc.scalar.activation(e[:], s4[:], AF.Exp, scale=SCALE)
                        E.append(e)
                    if debug and g == 0:
                        for tt in range(TT):
                            nc.sync.dma_start(
                                dbg["E0"][lh, tt * 128 : (tt + 1) * 128, :],
                                E[tt][:].bitcast(F32),
                            )
                    if stage < 3:
                        continue

                    ovs_g = []
                    sg = sg_ps.tile([128, 512], F32, tag="sg")
                    for hl in range(4):  # head local to group
                        ov = ov_ps.tile([128, 512], F32, tag="ov")
                        for tt in range(TT):
                            nc.tensor.matmul(
                                ov[0:34, :],
                                vaug[tt][:, 4 * g + hl, :],
                                E[tt][:, 512 * hl : 512 * (hl + 1)],
                                start=(tt == 0),
                                stop=(tt == TT - 1),
                            )
                        ovs = ovpool.tile([128, 512], F32R, tag="ovs")
                        nc.vector.tensor_copy(ovs[0:34, :], ov[0:34, :])
                        ovs_g.append(ovs)
                        if debug and g == 0:
                            nc.sync.dma_start(
                                dbg["ov"][hl, :, lh * 512 : (lh + 1) * 512],
                                ovs[0:34, :].bitcast(F32),
                            )
                        if stage < 4:
                            continue
                        # gather this head's sums row into partition 32*hl of sg
                        nc.tensor.matmul(
                            sg[:],
                            ind4[hl][:],
                            ovs[0:34, :],
                            start=(hl == 0),
                            stop=(hl == 3),
                        )
                    if stage < 4:
                        continue
                    # batched precise reciprocal of the 4 sums rows (other
                    # rows are zeros -> inf, never read)
                    rrf = rpool.tile([128, 512], F32, tag="rrf")
                    nc.vector.reciprocal(rrf[:], sg[:])
                    rr = rpool.tile([128, 512], F32R, tag="rr")
                    nc.vector.tensor_copy(rr[:], rrf[:])
                    for hl in range(4):
                        Rb = rb_ps.tile([128, 512], F32, tag="Rb")
                        nc.tensor.matmul(
                            Rb[0:32, :],
                            ones1[32 * hl : 32 * hl + 1, :],
                            rr[32 * hl : 32 * hl + 1, :],
                            start=True,
                            stop=True,
                            tile_position=(32 * hl, 0),
                        )
                        nc.vector.tensor_tensor(
                            attn_sb[g][
                                32 * hl : 32 * (hl + 1),
                                lh * 512 : (lh + 1) * 512,
                            ],
                            ovs_g[hl][0:32, :].bitcast(F32),
                            Rb[0:32, :],
                            ALU.mult,
                        )
                        if debug and g == 0:
                            rbc = rpool.tile([128, 512], F32, tag="rbc")
                            nc.vector.tensor_copy(rbc[0:32, :], Rb[0:32, :])
                            nc.sync.dma_start(
                                dbg["R"][
                                    32 * hl : 32 * (hl + 1),
                                    lh * 512 : (lh + 1) * 512,
                                ],
                                rbc[0:32, :],
                            )
            if debug and stage >= 4:
                for i in range(CT):
                    nc.sync.dma_start(
                        dbg["attn"][i * 128 : (i + 1) * 128, :],
                        attn_sb[i][:].bitcast(F32),
                    )

        # ---- output projection ----
        with ExitStack() as ph3:
            o_ps = ph3.enter_context(tc.tile_pool(name="o_ps", bufs=3, space="PSUM"))
            for ot in range(CT) if stage >= 5 else []:
                for th in range(TH):
                    op = o_ps.tile([128, 512], F32, tag="o")
                    for kt in range(CT):
                        nc.tensor.matmul(
                            op[:],
                            wT_sb["o"][kt][:, ot * 128 : (ot + 1) * 128],
                            attn_sb[kt][:, th * 512 : (th + 1) * 512],
                            start=(kt == 0),
                            stop=(kt == CT - 1),
                        )
                    oc = copies.tile([128, 512], F32, tag="oc")
                    nc.vector.tensor_copy(oc[:], op[:])
                    nc.sync.dma_start(
                        outT_d[ot * 128 : (ot + 1) * 128, th * 512 : (th + 1) * 512],
                        oc[:],
                    )

    nc.compile()
    return nc


def _prep_inputs(x, conv_w, bn_gamma, bn_beta, bn_mean, bn_var, wq, wk, wv, wo):
    f32 = np.float32
    inv = (bn_gamma / np.sqrt(bn_var + BN_EPS)).astype(f32)
    w9 = (conv_w.reshape(C, 9) * inv[:, None]).astype(f32)
    bias = (bn_beta - bn_mean * inv).astype(f32).reshape(C, 1)
    wqT = np.ascontiguousarray(np.asarray(wq, f32).T)
    wkT = np.ascontiguousarray(np.asarray(wk, f32).T)
    wvT = np.ascontiguousarray(np.asarray(wv, f32).T)
    woT = np.ascontiguousarray(np.asarray(wo, f32).T)
    maps = []
    for b in range(B):
        maps.append(
            {
                "xt": np.ascontiguousarray(np.asarray(x[b], f32).T),
                "w9": w9,
                "bias": bias,
                "wqT": wqT,
                "wkT": wkT,
                "wvT": wvT,
                "woT": woT,
            }
        )
    return maps


def kernel(x, conv_w, bn_gamma, bn_beta, bn_mean, bn_var, wq, wk, wv, wo, h, w,
           **kw):
    assert int(h) == HH and int(w) == WW
    from concourse.bass_utils import run_bass_kernel_spmd

    if "nc" not in _CACHE:
        _CACHE["nc"] = _build()
    nc = _CACHE["nc"]
    maps = _prep_inputs(
        x, conv_w, bn_gamma, bn_beta, bn_mean, bn_var, wq, wk, wv, wo
    )
    res = run_bass_kernel_spmd(nc, maps, list(range(NCORES)))
    out = np.stack([res.results[b]["outT"].T for b in range(B)])
    return out.astype(np.float32)


# revision 32
# speedup vs baseline: 1.0224x; 1.0224x over previous
"""Trainium2 Bass kernel for conv-projected multi-head attention.

Reference computation (per batch item b of 8, one NeuronCore each):
  y   = BN(depthwise3x3(x_b reshaped to [C,32,32]))      # q = k = v = y
  q/k/v = y @ w{q,k,v}^T  (heads: 12 x 32)
  att = softmax((q @ k^T) * sqrt(32))
  out = (att @ v) @ wo^T

Device layout is channel-major ("transposed"): xT [C=384, T=1024].
 - conv: 9 accumulating diag-matmuls on PE over a zero-padded [34x34] image
 - qT/kT [o, t] via lhsT=w^T; v kept [t, o] (augmented with a ones column
   so the PV matmul also yields the softmax denominators)
 - scores S^T[t, l] per head with K=32 contraction, 4 heads row-packed in
   the PE array via tile_position
 - exp on ACT directly from a 4-bank PSUM tile, output fp32r
 - PV: lhsT = vaug [t,34], rhs = E [t,l]; out rows 0-31 = O^T, row 32 = sums
 - normalize after PV (per-head reciprocal broadcast), concat, out-proj
All matmuls in float32r (~2e-4 rel err, full PE rate).
"""
import sys

sys.path.insert(0, "/opt/trn_rl_repo")
from contextlib import ExitStack

import numpy as np

B, T, C = 8, 1024, 384
NH, DH = 12, 32
HH = WW = 32
SCALE = float(DH) ** 0.5
BN_EPS = 1e-5
NCORES = 8

_CACHE = {}


def _build(debug=False, stage=5):
    import concourse.bass as bass
    import concourse.tile as tile
    from concourse import bacc, mybir
    from concourse.masks import make_identity

    F32 = mybir.dt.float32
    F32R = mybir.dt.float32r
    AF = mybir.ActivationFunctionType
    ALU = mybir.AluOpType

    nc = bacc.Bacc("TRN2", target_bir_lowering=False, debug=False)

    xt_d = nc.dram_tensor("xt", [C, T], F32R, kind="ExternalInput").ap()
    w9_d = nc.dram_tensor("w9", [C, 9], F32, kind="ExternalInput").ap()
    bias_d = nc.dram_tensor("bias", [C, 1], F32, kind="ExternalInput").ap()
    wqT_d = nc.dram_tensor("wqT", [C, C], F32R, kind="ExternalInput").ap()
    wkT_d = nc.dram_tensor("wkT", [C, C], F32R, kind="ExternalInput").ap()
    wvT_d = nc.dram_tensor("wvT", [C, C], F32R, kind="ExternalInput").ap()
    woT_d = nc.dram_tensor("woT", [C, C], F32R, kind="ExternalInput").ap()
    ind_d = nc.dram_tensor("ind", [4, 34, 128], F32R, kind="ExternalInput").ap()
    bind_d = nc.dram_tensor("bind", [4, 128], F32R, kind="ExternalInput").ap()
    outT_d = nc.dram_tensor("outT", [C, T], F32, kind="ExternalOutput").ap()
    dbg = {}
    if debug:
        dbg["y"] = nc.dram_tensor("dbg_y", [C, T], F32, kind="ExternalOutput").ap()
        dbg["qT"] = nc.dram_tensor("dbg_qT", [C, T], F32, kind="ExternalOutput").ap()
        dbg["vaug"] = nc.dram_tensor(
            "dbg_vaug", [T, NH * 34], F32, kind="ExternalOutput"
        ).ap()
        dbg["E0"] = nc.dram_tensor(
            "dbg_E0", [2, T, 4 * 512], F32, kind="ExternalOutput"
        ).ap()
        dbg["attn"] = nc.dram_tensor(
            "dbg_attn", [C, T], F32, kind="ExternalOutput"
        ).ap()
        dbg["ov"] = nc.dram_tensor(
            "dbg_ov", [4, 34, T], F32, kind="ExternalOutput"
        ).ap()
        dbg["R"] = nc.dram_tensor("dbg_R", [128, T], F32, kind="ExternalOutput").ap()

    CT = C // 128  # 3 c-tiles
    TT = T // 128  # 8 t-tiles
    TH = T // 512  # 2 t-halves / l-halves

    with tile.TileContext(nc) as tc, ExitStack() as top:
        # ---- persistent pools ----
        persist = top.enter_context(tc.tile_pool(name="persist", bufs=1))
        copies = top.enter_context(tc.tile_pool(name="copies", bufs=3))

        # persistent SBUF tensors
        y_sb = [persist.tile([128, T], F32R, tag=f"y{i}", name=f"y{i}") for i in range(CT)]
        qT_sb = [persist.tile([128, T], F32R, tag=f"q{i}", name=f"q{i}") for i in range(CT)]
        kT_sb = [persist.tile([128, T], F32R, tag=f"k{i}", name=f"k{i}") for i in range(CT)]
        vaug = [persist.tile([128, NH, 34], F32R, tag=f"va{i}", name=f"va{i}") for i in range(TT)]
        attn_sb = [persist.tile([128, T], F32R, tag=f"at{i}", name=f"at{i}") for i in range(CT)] if stage >= 4 else None
        wT_sb = {}
        for nm, d in (("q", wqT_d), ("k", wkT_d), ("v", wvT_d), ("o", woT_d)):
            tiles = [persist.tile([128, C], F32R, tag=f"w{nm}{i}", name=f"w{nm}{i}") for i in range(CT)]
            for i in range(CT):
                nc.sync.dma_start(tiles[i][:], d[i * 128 : (i + 1) * 128, :])
            wT_sb[nm] = tiles

        bias_sb = [persist.tile([128, 1], F32, tag=f"b{i}", name=f"b{i}") for i in range(CT)]
        for i in range(CT):
            nc.sync.dma_start(bias_sb[i][:], bias_d[i * 128 : (i + 1) * 128, :])

        ones1 = persist.tile([128, 32], F32R, tag="ones1")
        nc.vector.memset(ones1[:].bitcast(F32), 1.0)

        with ExitStack() as ph1:
            convpool = ph1.enter_context(tc.tile_pool(name="convpool", bufs=1))
            conv_ps = ph1.enter_context(
                tc.tile_pool(name="conv_ps", bufs=2, space="PSUM")
            )
            qk_ps = ph1.enter_context(tc.tile_pool(name="qk_ps", bufs=4, space="PSUM"))
            v_ps = ph1.enter_context(tc.tile_pool(name="v_ps", bufs=2, space="PSUM"))

            # ---- padded input and diag weights ----
            xp = [convpool.tile([128, 34 * 34], F32R, tag=f"xp{i}", name=f"xp{i}") for i in range(CT)]
            w9_sb = [convpool.tile([128, 9], F32, tag=f"w9{i}", name=f"w9s{i}") for i in range(CT)]
            ident = convpool.tile([128, 128], F32, tag="ident")
            diag = [convpool.tile([128, 9, 128], F32R, tag=f"dg{i}", name=f"dg{i}") for i in range(CT)]

            make_identity(nc, ident[:])
            for i in range(CT):
                nc.vector.memset(xp[i][:].bitcast(F32), 0.0)
                nc.sync.dma_start(
                    xp[i][:].rearrange("p (a b) -> p a b", a=34)[:, 1:33, 1:33],
                    xt_d[i * 128 : (i + 1) * 128, :].rearrange(
                        "p (a b) -> p a b", a=32
                    ),
                )
                nc.sync.dma_start(w9_sb[i][:], w9_d[i * 128 : (i + 1) * 128, :])
                for k in range(9):
                    nc.vector.tensor_scalar_mul(
                        diag[i][:, k, :], ident[:], w9_sb[i][:, k : k + 1]
                    )

            # ---- conv: 9 accumulating diag matmuls per (c-tile, t-half) ----
            for i in range(CT):
                for th in range(TH):
                    yp = conv_ps.tile([128, 512], F32, tag="conv")
                    r0 = th * 16  # image-row offset of this half
                    for k in range(9):
                        dy, dx = k // 3 - 1, k % 3 - 1
                        off = (r0 + 1 + dy) * 34 + (1 + dx)
                        rhs = bass.AP(
                            tensor=xp[i].tensor,
                            offset=xp[i].offset + off,
                            ap=[list(p) for p in xp[i].ap[:1]] + [[34, 16], [1, 32]],
                        )
                        nc.tensor.matmul(
                            yp[:].rearrange("p (a b) -> p a b", a=16),
                            diag[i][:, k, :],
                            rhs,
                            start=(k == 0),
                            stop=(k == 8),
                        )
                    # + BN bias, round to f32r, store to y
                    nc.vector.tensor_scalar_add(
                        y_sb[i][:, th * 512 : (th + 1) * 512],
                        yp[:],
                        bias_sb[i][:],
                    )
            if debug:
                for i in range(CT):
                    nc.sync.dma_start(
                        dbg["y"][i * 128 : (i + 1) * 128, :], y_sb[i][:].bitcast(F32)
                    )

            # ---- q/k projections: qT[o, t] ----
            # o-tile outer so scores for group g can start once both q and k
            # of that group are done (attention doesn't wait for all of qk)
            for ot in range(CT):
                for nm, dst in (("q", qT_sb), ("k", kT_sb)):
                    for th in range(TH):
                        pp = qk_ps.tile([128, 512], F32, tag="qk")
                        for kt in range(CT):
                            nc.tensor.matmul(
                                pp[:],
                                wT_sb[nm][kt][:, ot * 128 : (ot + 1) * 128],
                                y_sb[kt][:, th * 512 : (th + 1) * 512],
                                start=(kt == 0),
                                stop=(kt == CT - 1),
                            )
                        nc.vector.tensor_copy(
                            dst[ot][:, th * 512 : (th + 1) * 512], pp[:]
                        )
            if debug:
                for i in range(CT):
                    nc.sync.dma_start(
                        dbg["qT"][i * 128 : (i + 1) * 128, :], qT_sb[i][:].bitcast(F32)
                    )

            # ---- v projection: v[t, o], written into vaug [t, h, 34] ----
            for tt in range(TT):
                vp = v_ps.tile([128, C], F32, tag="v")
                for kt in range(CT):
                    nc.tensor.matmul(
                        vp[:],
                        y_sb[kt][:, tt * 128 : (tt + 1) * 128],
                        wT_sb["v"][kt][:],
                        start=(kt == 0),
                        stop=(kt == CT - 1),
                    )
                nc.vector.memset(vaug[tt][:, :, 32:34].bitcast(F32), 0.0)
                nc.vector.memset(vaug[tt][:, :, 32:33].bitcast(F32), 1.0)
                nc.vector.tensor_copy(
                    vaug[tt][:, :, 0:32],
                    vp[:].rearrange("p (h d) -> p h d", h=NH),
                )
            if debug:
                for tt in range(TT):
                    nc.sync.dma_start(
                        dbg["vaug"][tt * 128 : (tt + 1) * 128, :],
                        vaug[tt][:].bitcast(F32).rearrange("p a b -> p (a b)"),
                    )

        # ---- attention ----
        with ExitStack() as ph2:
            s_ps = ph2.enter_context(tc.tile_pool(name="s_ps", bufs=1, space="PSUM"))
            ov_ps = ph2.enter_context(tc.tile_pool(name="ov_ps", bufs=2, space="PSUM"))
            sg_ps = ph2.enter_context(tc.tile_pool(name="sg_ps", bufs=1, space="PSUM"))
            rb_ps = ph2.enter_context(tc.tile_pool(name="rb_ps", bufs=1, space="PSUM"))
            epool = ph2.enter_context(tc.tile_pool(name="epool", bufs=10))
            rpool = ph2.enter_context(tc.tile_pool(name="rpool", bufs=3))
            ovpool = ph2.enter_context(tc.tile_pool(name="ovpool", bufs=6))

            # indicator tiles (host-built): ind4[j][k, m] = 1.0 iff
            # (k, m) == (32, j) — the gather matmul moves a head's sums row
            # (psum partition 32) to partition j of the gather tile.
            # bind[k, m] = 1.0 iff m // 32 == k — the K=4 broadcast matmul
            # bind.T @ rr[0:4] replicates head k's recip row across psum
            # partitions 32k..32k+31 for all 4 heads at once.
            ind4 = []
            for j in range(4):
                it = rpool.tile([34, 128], F32R, tag=f"ind{j}", name=f"ind{j}")
                nc.sync.dma_start(it[:], ind_d[j])
                ind4.append(it)
            bind = rpool.tile([4, 128], F32R, tag="bind", name="bind")
            nc.sync.dma_start(bind[:], bind_d)

            for lh in range(TH):
                for g in range(CT):
                    E = []
                    for tt in range(TT):
                        s4 = s_ps.tile([128, 2048], F32, tag="s4")
                        for hh in range(4):
                            nc.tensor.matmul(
                                s4[:, 512 * hh : 512 * (hh + 1)],
                                kT_sb[g][
                                    32 * hh : 32 * (hh + 1),
                                    tt * 128 : (tt + 1) * 128,
                                ],
                                qT_sb[g][
                                    32 * hh : 32 * (hh + 1),
                                    lh * 512 : (lh + 1) * 512,
                                ],
                                start=True,
                                stop=True,
                                tile_position=(32 * hh, 0),
                            )
                        e = epool.tile([128, 2048], F32R, tag="E")
                        nc.scalar.activation(e[:], s4[:], AF.Exp, scale=SCALE)
                        E.append(e)
                    if debug and g == 0:
                        for tt in range(TT):
                            nc.sync.dma_start(
                                dbg["E0"][lh, tt * 128 : (tt + 1) * 128, :],
                                E[tt][:].bitcast(F32),
                            )
                    if stage < 3:
                        continue

                    ovs_g = []
                    sg = sg_ps.tile([128, 512], F32, tag="sg")
                    for hl in range(4):  # head local to group
                        ov = ov_ps.tile([128, 512], F32, tag="ov")
                        for tt in range(TT):
                            nc.tensor.matmul(
                                ov[0:34, :],
                                vaug[tt][:, 4 * g + hl, :],
                                E[tt][:, 512 * hl : 512 * (hl + 1)],
                                start=(tt == 0),
                                stop=(tt == TT - 1),
                            )
                        ovs = ovpool.tile([128, 512], F32R, tag="ovs")
                        nc.vector.tensor_copy(ovs[0:34, :], ov[0:34, :])
                        ovs_g.append(ovs)
                        if debug and g == 0:
                            nc.sync.dma_start(
                                dbg["ov"][hl, :, lh * 512 : (lh + 1) * 512],
                                ovs[0:34, :].bitcast(F32),
                            )
                        if stage < 4:
                            continue
                        # gather this head's sums row into partition 32*hl of sg
                        nc.tensor.matmul(
                            sg[:],
                            ind4[hl][:],
                            ovs[0:34, :],
                            start=(hl == 0),
                            stop=(hl == 3),
                        )
                    if stage < 4:
                        continue
                    # batched precise reciprocal of the 4 sums rows (other
                    # rows are zeros -> inf, never read)
                    rrf = rpool.tile([128, 512], F32, tag="rrf")
                    nc.vector.reciprocal(rrf[0:4, :], sg[0:4, :])
                    rr = rpool.tile([128, 512], F32R, tag="rr")
                    nc.vector.tensor_copy(rr[0:4, :], rrf[0:4, :])
                    # one K=4 matmul broadcasts all 4 heads' recips to
                    # partitions 32*hl .. 32*hl+31
                    Rb = rb_ps.tile([128, 512], F32, tag="Rb")
                    nc.tensor.matmul(
                        Rb[:], bind[:], rr[0:4, :], start=True, stop=True
                    )
                    for hl in range(4):
                        nc.vector.tensor_tensor(
                            attn_sb[g][
                                32 * hl : 32 * (hl + 1),
                                lh * 512 : (lh + 1) * 512,
                            ],
                            ovs_g[hl][0:32, :].bitcast(F32),
                            Rb[32 * hl : 32 * (hl + 1), :],
                            ALU.mult,
                        )
                        if debug and g == 0:
                            rbc = rpool.tile([128, 512], F32, tag="rbc")
                            nc.vector.tensor_copy(
                                rbc[0:32, :], Rb[32 * hl : 32 * (hl + 1), :]
                            )
                            nc.sync.dma_start(
                                dbg["R"][
                                    32 * hl : 32 * (hl + 1),
                                    lh * 512 : (lh + 1) * 512,
                                ],
                                rbc[0:32, :],
                            )
            if debug and stage >= 4:
                for i in range(CT):
                    nc.sync.dma_start(
                        dbg["attn"][i * 128 : (i + 1) * 128, :],
                        attn_sb[i][:].bitcast(F32),
                    )

        # ---- output projection ----
        with ExitStack() as ph3:
            o_ps = ph3.enter_context(tc.tile_pool(name="o_ps", bufs=3, space="PSUM"))
            for ot in range(CT) if stage >= 5 else []:
                for th in range(TH):
                    op = o_ps.tile([128, 512], F32, tag="o")
                    for kt in range(CT):
                        nc.tensor.matmul(
                            op[:],
                            wT_sb["o"][kt][:, ot * 128 : (ot + 1) * 128],
                            attn_sb[kt][:, th * 512 : (th + 1) * 512],
                            start=(kt == 0),
                            stop=(kt == CT - 1),
                        )
                    oc = copies.tile([128, 512], F32, tag="oc")
                    nc.vector.tensor_copy(oc[:], op[:])
                    nc.sync.dma_start(
                        outT_d[ot * 128 : (ot + 1) * 128, th * 512 : (th + 1) * 512],
                        oc[:],
                    )

    nc.compile()
    return nc


def _prep_inputs(x, conv_w, bn_gamma, bn_beta, bn_mean, bn_var, wq, wk, wv, wo):
    f32 = np.float32
    inv = (bn_gamma / np.sqrt(bn_var + BN_EPS)).astype(f32)
    w9 = (conv_w.reshape(C, 9) * inv[:, None]).astype(f32)
    bias = (bn_beta - bn_mean * inv).astype(f32).reshape(C, 1)
    wqT = np.ascontiguousarray(np.asarray(wq, f32).T)
    wkT = np.ascontiguousarray(np.asarray(wk, f32).T)
    wvT = np.ascontiguousarray(np.asarray(wv, f32).T)
    woT = np.ascontiguousarray(np.asarray(wo, f32).T)
    ind = np.zeros((4, 34, 128), f32)
    for j in range(4):
        ind[j, 32, j] = 1.0
    bind = np.zeros((4, 128), f32)
    for j in range(4):
        bind[j, 32 * j : 32 * (j + 1)] = 1.0
    maps = []
    for b in range(B):
        maps.append(
            {
                "xt": np.ascontiguousarray(np.asarray(x[b], f32).T),
                "w9": w9,
                "bias": bias,
                "wqT": wqT,
                "wkT": wkT,
                "wvT": wvT,
                "woT": woT,
                "ind": ind,
                "bind": bind,
            }
        )
    return maps


def kernel(x, conv_w, bn_gamma, bn_beta, bn_mean, bn_var, wq, wk, wv, wo, h, w,
           **kw):
    assert int(h) == HH and int(w) == WW
    from concourse.bass_utils import run_bass_kernel_spmd

    if "nc" not in _CACHE:
        _CACHE["nc"] = _build()
    nc = _CACHE["nc"]
    maps = _prep_inputs(
        x, conv_w, bn_gamma, bn_beta, bn_mean, bn_var, wq, wk, wv, wo
    )
    res = run_bass_kernel_spmd(nc, maps, list(range(NCORES)))
    out = np.stack([res.results[b]["outT"].T for b in range(B)])
    return out.astype(np.float32)
